# revision 14
# baseline (speedup 1.0000x reference)
"""Batched forward-kinematics (DiffKin) Bass kernel for 8 TRN2 NeuronCores, v2.

Problem (hardcoded): B=65536 configurations, 32-frame kinematic tree, 29 DOF,
PARENTS = [-1, 0..28, 10, 10]. Output (B, 32, 4, 4) fp32 poses.

Design (vs v1 baseline): component-major fp32 layout. Every SBUF tile is
(P=128 partitions = batch rows, C components, Q=64 batch columns) so every
DVE operand iterates innermost over Q with stride 1 (measured: fp32
tensor_tensor = (FD+151)/0.96GHz; the baseline's q-innermost-stride-16 APs
ran ~3x slower). Engine split measured-driven:
 - DVE: local-matrix giant products + the sequential compose chain.
 - ACT (scalar): per-frame angle affines, sin/cos (arg wrapped to [-pi,pi]
   via fp32 magic rounding on DVE; cos = sin(pi/2 - |y|)).
 - GPSIMD issues SWDGE accumulate-DMAs (dst += src) for the giant local
   merge-adds, overlapping the DVE compose chain (direct GPSIMD arithmetic
   measured to contend destructively with DVE on the shared SBUF port).
 - Revolute locals have a constant translation column: rotation is composed
   with 5 FD=576 ops, translation with 3 FD=192 scalar_tensor_tensor ops
   using immediate constants.
Frame 0's pose is constant: the host fills it (and all bottom rows) directly;
the device outputs 12 components (3x4) per remaining frame.

fp16 was measured 2x faster per op but is numerically dead here: the error
gate is entrywise with a 1e-3 scale floor, so abs error must stay ~2e-5 on
O(1) intermediates -> fp32 mantissa required.
"""
import numpy as np
from contextlib import ExitStack

import concourse.bass as bass
import concourse.mybir as mybir
from concourse.bass_utils import run_bass_kernel_spmd

FP = mybir.dt.float32
Alu = mybir.AluOpType
ActF = mybir.ActivationFunctionType

B = 65536
NFRAMES = 32
DOF = 29
NCORES = 8
BC = B // NCORES
P = 128
Q = BC // P
PARENTS = [-1] + list(range(29)) + [10, 10]
ORDER = list(range(11)) + [30, 31] + list(range(11, 30))

MAGIC = float(1.5 * 2 ** 23)
INV2PI = float(1.0 / (2 * np.pi))
TWOPI = float(2 * np.pi)
HALFPI = float(np.pi / 2)

NPOSE = 6
NCHUNK = 4          # comp-trig giant chunks
ACCUM_DMA = True    # G3/G4 merge-adds via SWDGE accumulate-DMA (else DVE)

_cache = {}


def _skew(a):
    return np.array([[0, -a[2], a[1]], [a[2], 0, -a[0]], [-a[1], a[0], 0]],
                    dtype=np.float64)


def _plan_host(all_axes, all_origins, mimic_multipliers, mimic_offsets,
               ctrlable_indices, mimic_src_indices, mimic_dst_indices,
               joint_types):
    """Fold all structural inputs into per-frame nodes.

    Returns (nodes, ok) where ok=False means an assumption of the fast path
    failed (non-affine origins) and the caller must use the fallback kernel.
    Node kinds: 'cpose' (constant pose, host-filled), 'mat' (variable local,
    constant parent -> direct pose materialization), 'comp' (compose), each
    trig (M0 + sin*M1 + cos*M2) or linear (M0 + x*M1) or const local.
    """
    axes = np.asarray(all_axes, np.float64)
    origins = np.asarray(all_origins, np.float64)
    mm = np.asarray(mimic_multipliers, np.float64)
    mo = np.asarray(mimic_offsets, np.float64)
    ctrl = np.asarray(ctrlable_indices, np.int64)
    msrc = np.asarray(mimic_src_indices, np.int64)
    mdst = np.asarray(mimic_dst_indices, np.int64)
    types = np.asarray(joint_types, np.int64)

    if not np.all(origins[:, 3, :] == np.array([0, 0, 0, 1.0])):
        return None, False

    amap = {f: (None, 0.0, 0.0) for f in range(NFRAMES)}
    for d, f in enumerate(ctrl):
        amap[int(f)] = (d, 1.0, 0.0)
    ctrl_only = dict(amap)
    for j in range(len(mdst)):
        s, dcol = int(msrc[j]), int(mdst[j])
        scol, ssc, soff = ctrl_only[s]
        if scol is None:
            amap[dcol] = (None, 0.0, float(mm[j] * soff + mo[j]))
        else:
            amap[dcol] = (scol, float(mm[j] * ssc), float(mm[j] * soff + mo[j]))

    I4 = np.eye(4)

    def local_decomp(f):
        O = origins[f]
        col, sc, off = amap[f]
        t = int(types[f])
        a = axes[f]
        n2 = float(a @ a)
        if t == 1 and n2 > 1e-24:
            n = np.sqrt(n2)
            S = np.zeros((4, 4)); S[:3, :3] = _skew(a)
            S2 = S @ S
            M1 = O @ S / n
            M2n = O @ S2 / n2
            if col is None:
                phi = n * off
                return ('const', O + np.sin(phi) * M1 + (1 - np.cos(phi)) * M2n)
            return ('rev', O + M2n, M1, -M2n, n * sc, n * off)
        if t == 2 and n2 > 0:
            A = np.zeros((4, 4)); A[:3, 3] = a
            M1 = O @ A
            if col is None:
                return ('const', O + off * M1)
            return ('prism', O, M1, sc, off)
        return ('const', O)

    nodes = []
    pose_const = {}
    for f in ORDER:
        p = PARENTS[f]
        Cp = I4 if p < 0 else pose_const.get(p)
        dec = local_decomp(f)
        if dec[0] == 'const':
            L = dec[1]
            if Cp is not None:
                M = Cp @ L
                pose_const[f] = M
                nodes.append(dict(kind='cpose', f=f, M=M))
            else:
                nodes.append(dict(kind='ccomp', f=f, parent=p, M0=L))
        elif dec[0] == 'rev':
            _, M0, M1, M2, sc, off = dec
            kw = dict(f=f, col=amap[f][0], sc=sc, off=off, trig=True, parent=p)
            if Cp is not None:
                nodes.append(dict(kind='mat', M0=Cp @ M0, M1=Cp @ M1,
                                  M2=Cp @ M2, **kw))
            else:
                nodes.append(dict(kind='comp', M0=M0, M1=M1, M2=M2, **kw))
        else:
            _, M0, M1, sc, off = dec
            kw = dict(f=f, col=amap[f][0], sc=sc, off=off, trig=False, parent=p)
            if Cp is not None:
                nodes.append(dict(kind='mat', M0=Cp @ M0, M1=Cp @ M1, M2=None,
                                  **kw))
            else:
                nodes.append(dict(kind='comp', M0=M0, M1=M1, M2=None, **kw))
    return nodes, True


def _build_program(nodes):
    """Emit the Bass program. Returns (nc, cb_row, imgs, meta)."""
    # ---- classify ----
    comp_trig = [nd for nd in nodes if nd['kind'] == 'comp' and nd.get('trig')]
    NTC = len(comp_trig)
    # chunking for the giant local ops
    nchunk = min(NCHUNK, max(NTC, 1))
    bounds = [NTC * i // nchunk for i in range(nchunk + 1)]
    for i, nd in enumerate(comp_trig):
        nd['t'] = i                       # trig row in phi/sin/cos
        nd['g'] = i                       # loc tile row
        nd['chunk'] = next(c for c in range(nchunk) if bounds[c] <= i < bounds[c + 1])
    mat_trig = [nd for nd in nodes if nd['kind'] == 'mat' and nd.get('trig')]
    for j, nd in enumerate(mat_trig):
        nd['t'] = NTC + j
    NT = NTC + len(mat_trig)
    lin_nodes = [nd for nd in nodes if nd['kind'] in ('mat', 'comp')
                 and not nd.get('trig', True)]
    for j, nd in enumerate(lin_nodes):
        nd['l'] = j
    NL = len(lin_nodes)

    # ---- constants: cb row + DRAM accumulate images ----
    cb = []

    def cpush(vec):
        off = len(cb)
        cb.extend(float(x) for x in vec)
        return off

    def rot9(M):
        return np.asarray(M, np.float64)[:3, :3].reshape(9)

    # comp-trig first: giants need contiguous per-chunk 18-float B|C blocks
    for nd in comp_trig:
        nd['lt'] = [float(nd['M0'][k, 3]) for k in range(3)]
        nd['c_B'] = cpush(rot9(nd['M1']))
        nd['c_C'] = cpush(rot9(nd['M2']))
    if not ACCUM_DMA:
        for nd in comp_trig:
            nd['c_M0'] = cpush(rot9(nd['M0']))
    for nd in nodes:
        if nd['kind'] in ('mat', 'comp'):
            nd['c_off'] = cpush([nd['off']])
    c_halfpi = cpush([HALFPI])
    for nd in nodes:
        if nd['kind'] == 'cpose' or (nd['kind'] == 'comp' and nd.get('trig')):
            continue
        if nd['kind'] == 'ccomp':
            nd['c_R'] = cpush(rot9(nd['M0']))
            nd['lt'] = [float(nd['M0'][k, 3]) for k in range(3)]
            continue
        if nd.get('trig'):
            nd['lt'] = [float(nd['M0'][k, 3]) for k in range(3)]
            nd['c_B'] = cpush(rot9(nd['M1']))
            nd['c_C'] = cpush(rot9(nd['M2']))
            nd['c_M0'] = cpush(rot9(nd['M0']))
            nd['c_t'] = cpush(nd['lt'])
        else:
            # linear (prismatic): rotation const = M0 rot; translation
            # = M0t + x * M1t (M1 rotation part is zero).
            nd['c_R'] = cpush(rot9(nd['M0']))
            nd['m0t'] = [float(nd['M0'][k, 3]) for k in range(3)]
            nd['a3'] = [float(nd['M1'][k, 3]) for k in range(3)]
            nd['c_m0t'] = cpush(nd['m0t'])
            if nd['kind'] == 'mat':
                nd['c_t'] = cpush(nd['m0t'])  # only valid if x-independent; not used
    NCB = max(len(cb), 4)
    cb_row = np.zeros(NCB, np.float32)
    cb_row[:len(cb)] = np.asarray(cb, np.float32)

    # M0 rotation image for the G4 accumulate-DMA: (NTC, 9, Q) repeated per q
    m0img = np.zeros((max(NTC, 1), 9, Q), np.float32)
    for nd in comp_trig:
        m0img[nd['g'], :, :] = rot9(nd['M0']).astype(np.float32)[:, None]

    # ---- device sequence: mat nodes then comp nodes, in ORDER ----
    seq = [nd for nd in nodes if nd['kind'] in ('mat', 'comp', 'ccomp')]
    frame_to_node = {nd['f']: nd for nd in seq}
    const_pose = {nd['f']: nd['M'] for nd in nodes if nd['kind'] == 'cpose'}
    for oi, nd in enumerate(seq):
        nd['oi'] = oi
        nd['buf'] = oi % NPOSE

    def prior_uses(nd):
        return sum(1 for n2 in seq[:nd['oi']] if n2['buf'] == nd['buf'])

    # ---- bass program ----
    nc = bass.Bass()
    ja_in = nc.declare_dram_parameter("ja", [P, DOF, Q], FP, isOutput=False)
    cb_in = nc.declare_dram_parameter("cb", [P, NCB], FP, isOutput=False)
    m0_in = nc.declare_dram_parameter("m0img", [P, max(NTC, 1) * 9 * Q], FP,
                                      isOutput=False)
    out_d = nc.declare_dram_parameter("out", [len(seq), P, 12 * Q], FP,
                                      isOutput=True)

    NTx = max(NT, 1)
    NTCx = max(NTC, 1)
    NLx = max(NL, 1)
    chunk_tmp_rows = max((bounds[c + 1] - bounds[c] for c in range(nchunk)),
                         default=1)

    with ExitStack() as st:
        def sb(name, shape):
            return st.enter_context(nc.sbuf_tensor(name, shape, FP))

        ja = sb("ja_t", [P, DOF, Q])
        cbt = sb("cb_t", [P, NCB])
        phi = sb("phi_t", [P, NTx, Q])
        kt = sb("k_t", [P, NTx, Q])
        yt = sb("y_t", [P, NTx, Q])
        ayt = sb("ay_t", [P, NTx, Q])
        sinv = sb("sin_t", [P, NTx, Q])
        cosv = sb("cos_t", [P, NTx, Q])
        xlin = sb("xlin_t", [P, NLx, Q])
        ut = sb("u_t", [P, NLx * 3, Q])
        locs = sb("locs_t", [P, NTCx, 9, Q])
        tmps = [sb(f"tmp{i}", [P, chunk_tmp_rows, 9, Q]) for i in range(2)]
        tA = sb("tA_t", [P, 9, Q])
        tB = sb("tB_t", [P, 9, Q])
        poses = [sb(f"pose{i}", [P, 12, Q]) for i in range(NPOSE)]

        in_sem = st.enter_context(nc.semaphore(name="in_sem"))
        v_sem = st.enter_context(nc.semaphore(name="v_sem"))
        g_sem = st.enter_context(nc.semaphore(name="g_sem"))
        a_sem = st.enter_context(nc.semaphore(name="a_sem"))
        pd_sems = [st.enter_context(nc.semaphore(name=f"pd{i}"))
                   for i in range(NPOSE)]
        block = st.enter_context(nc.Block())

        # ---------- tick planning (host-side mirrors of each engine) ----------
        # ACT: per-chunk (sin, abs, cos) groups, then mat-trig rows group,
        # then 3*NL u-rows (affines run on DVE)
        n_groups = nchunk + (1 if len(mat_trig) else 0)
        a_groups_end = 3 * n_groups
        a_u = {nd['l']: a_groups_end + 3 * nd['l'] + 3 for nd in lin_nodes}

        # DVE tick bookkeeping
        vt = dict(n=0)

        def plan():
            vt['n'] += 1
            return vt['n']

        # DVE prelude: chunk0 affines + 3 range ops on chunk0 rows, then the
        # remaining affines + 3 range ops on the rest (lets ACT start early)
        n_c0 = bounds[1] if NTC else 0
        a_y0 = (n_c0 + 3) if NT else 0          # DVE tick when chunk0 y ready
        n_prelude = (NT + NL + 6) if NT else NL
        vt['n'] = n_prelude
        # giant locals: per chunk 2 mults (+2 adds if not ACCUM_DMA)
        g_mul_ticks = []
        for c in range(nchunk):
            t0 = plan(); t1 = plan()
            if not ACCUM_DMA:
                plan(); plan()
            g_mul_ticks.append((t0, t1))
        giants_end = vt['n']
        # per-node compose/materialize ticks
        for nd in seq:
            if nd['kind'] == 'mat':
                if nd.get('trig'):
                    for _ in range(4):
                        plan()
                    nd['v_done'] = vt['n']
                    plan()  # translation copy
                    nd['v_done'] = vt['n']
                else:
                    plan()  # rot copy from cb
                    for _ in range(3):
                        plan()  # translation u-copies
                    nd['v_done'] = vt['n']
            elif nd['kind'] == 'ccomp' or not nd.get('trig', True):
                n_ops = 5 + (6 if (nd['kind'] == 'comp' and not nd.get('trig', True))
                             else 3)
                for _ in range(n_ops):
                    plan()
                nd['v_done'] = vt['n']
            else:
                for _ in range(8):
                    plan()
                nd['v_done'] = vt['n']

        # ---------------- sync: input + output DMA ----------------
        @block.sync
        def _(sync):
            sync.dma_start(out=ja[:], in_=ja_in[:]).then_inc(in_sem, 16)
            sync.dma_start(out=cbt[:], in_=cb_in[:]).then_inc(in_sem, 16)
            uses = [0] * NPOSE
            for nd in seq:
                b = nd['buf']
                sync.wait_ge(v_sem, nd['v_done'])
                sync.dma_start(
                    out=out_d[nd['oi']],
                    in_=poses[b][:].rearrange("p c q -> p (c q)"),
                ).then_inc(pd_sems[b], 16)
                uses[b] += 1
            for i in range(NPOSE):
                if uses[i]:
                    sync.wait_ge(pd_sems[i], 16 * uses[i])

        # ---------------- gpsimd: accumulate-DMAs for G3/G4 ----------------
        if ACCUM_DMA and NTC:
            @block.gpsimd
            def _(gpsimd):
                done = 0
                for c in range(nchunk):
                    r0, r1 = bounds[c], bounds[c + 1]
                    nr = r1 - r0
                    if nr == 0:
                        continue
                    gpsimd.wait_ge(v_sem, g_mul_ticks[c][1])
                    # G3: locs[chunk] += tmp (cos*C part)
                    gpsimd.dma_start(
                        out=locs[:, r0:r1, :, :], in_=tmps[c % 2][:, 0:nr, :, :],
                        accum_op=Alu.add).then_inc(g_sem, 16)
                    done += 16
                    gpsimd.wait_ge(g_sem, done)
                    # G4: locs[chunk] += M0 image from DRAM
                    gpsimd.dma_start(
                        out=locs[:, r0:r1, :, :],
                        in_=m0_in[:, r0 * 9 * Q: r1 * 9 * Q]
                            .rearrange("p (g c q) -> p g c q", c=9, q=Q),
                        accum_op=Alu.add).then_inc(g_sem, 16)
                    done += 16
                    gpsimd.wait_ge(g_sem, done)

        # ---------------- scalar (ACT): affines, sin/cos, u rows ----------------
        @block.scalar
        def _(scalar):
            scalar.wait_ge(in_sem, 32)
            if NT:
                groups = [(bounds[c], bounds[c + 1]) for c in range(nchunk)]
                if len(mat_trig):
                    groups.append((NTC, NT))
                scalar.wait_ge(v_sem, a_y0)          # chunk0 y ready
                for gi, (r0, r1) in enumerate(groups):
                    if gi == 1:
                        scalar.wait_ge(v_sem, n_prelude)
                    if r0 == r1:
                        r1 = r0 + 1  # keep tick count; reuse row
                    nc.scalar.activation(out=sinv[:, r0:r1, :],
                                         in_=yt[:, r0:r1, :],
                                         func=ActF.Sin, bias=0.0, scale=1.0
                                         ).then_inc(a_sem, 1)
                    nc.scalar.activation(out=ayt[:, r0:r1, :],
                                         in_=yt[:, r0:r1, :],
                                         func=ActF.Abs, bias=0.0, scale=1.0
                                         ).then_inc(a_sem, 1)
                    nc.scalar.activation(out=cosv[:, r0:r1, :],
                                         in_=ayt[:, r0:r1, :],
                                         func=ActF.Sin,
                                         bias=cbt[:, c_halfpi:c_halfpi + 1],
                                         scale=-1.0).then_inc(a_sem, 1)
            if lin_nodes:
                scalar.wait_ge(v_sem, n_prelude)  # xlin ready
            for nd in lin_nodes:
                for k in range(3):
                    nc.scalar.activation(
                        out=ut[:, 3 * nd['l'] + k, :], in_=xlin[:, nd['l'], :],
                        func=ActF.Identity,
                        bias=cbt[:, nd['c_m0t'] + k:nd['c_m0t'] + k + 1],
                        scale=float(nd['a3'][k])).then_inc(a_sem, 1)

        # ---------------- vector: trig prelude, giants, composes ----------------
        @block.vector
        def _(vector):
            state = dict(v=0)

            def op(inst):
                inst.then_inc(v_sem, 1)
                state['v'] += 1
                return state['v']

            VW = False  # self-waits: DVE in-order + DRAIN make these redundant
            def vwait(n):
                if VW:
                    vector.wait_ge(v_sem, n)

            # --- angle affines + range reduction (all DVE, chunk0 first) ---
            vector.wait_ge(in_sem, 32)
            trig_all = comp_trig + mat_trig

            def affine(nd, tile, row):
                op(nc.vector.tensor_scalar(
                    out=tile[:, row, :], in0=ja[:, nd['col'], :],
                    scalar1=float(nd['sc']), scalar2=float(nd['off']),
                    op0=Alu.mult, op1=Alu.add))

            def range_red(r0, r1):
                PHI = phi[:, r0:r1, :]
                op(nc.vector.tensor_scalar(out=kt[:, r0:r1, :], in0=PHI,
                   scalar1=INV2PI, scalar2=MAGIC, op0=Alu.mult, op1=Alu.add))
                op(nc.vector.tensor_scalar_add(out=kt[:, r0:r1, :],
                   in0=kt[:, r0:r1, :], scalar1=-MAGIC))
                op(nc.vector.scalar_tensor_tensor(out=yt[:, r0:r1, :],
                   in0=kt[:, r0:r1, :], scalar=-TWOPI, in1=PHI,
                   op0=Alu.mult, op1=Alu.add))

            for nd in trig_all[:n_c0]:
                affine(nd, phi, nd['t'])
            if NT:
                range_red(0, n_c0)
                assert state['v'] == a_y0
            for nd in trig_all[n_c0:]:
                affine(nd, phi, nd['t'])
            for nd in lin_nodes:
                affine(nd, xlin, nd['l'])
            if NT:
                range_red(n_c0, NT)
            assert state['v'] == n_prelude

            # --- giant local products (per-chunk sin/cos waits) ---
            for c in range(nchunk):
                r0, r1 = bounds[c], bounds[c + 1]
                nr = r1 - r0
                if nr == 0:
                    continue
                ndl = comp_trig[r0:r1]
                b_off = ndl[0]['c_B']
                c_off = ndl[0]['c_C']
                # cb rows for a chunk are contiguous 18-float blocks per node:
                # B at c_B, C at c_C = c_B + 9, stride 18 between nodes.
                assert all(nd['c_B'] == b_off + 18 * i for i, nd in enumerate(ndl))
                assert all(nd['c_C'] == c_off + 18 * i for i, nd in enumerate(ndl))
                if ACCUM_DMA and c >= 2:
                    # tmp buffer reused from chunk c-2: its G3 accum must be done
                    vector.wait_ge(g_sem, 32 * (c - 1))
                vector.wait_ge(a_sem, 3 * c + 1)   # chunk sin ready
                op(nc.vector.tensor_tensor(
                    out=locs[:, r0:r1, :, :],
                    in0=sinv[:, r0:r1, :].unsqueeze(2).broadcast_to([P, nr, 9, Q]),
                    in1=cbt[:, b_off:b_off + 18 * nr]
                        .rearrange("p (g c) -> p g c", c=18)[:, :, 0:9]
                        .unsqueeze(3).broadcast_to([P, nr, 9, Q]),
                    op=Alu.mult))
                t0 = state['v']
                vwait(t0)
                vector.wait_ge(a_sem, 3 * c + 3)   # chunk cos ready
                op(nc.vector.tensor_tensor(
                    out=tmps[c % 2][:, 0:nr, :, :],
                    in0=cosv[:, r0:r1, :].unsqueeze(2).broadcast_to([P, nr, 9, Q]),
                    in1=cbt[:, c_off:c_off + 18 * nr]
                        .rearrange("p (g c) -> p g c", c=18)[:, :, 0:9]
                        .unsqueeze(3).broadcast_to([P, nr, 9, Q]),
                    op=Alu.mult))
                t1 = state['v']
                vwait(t1)
                assert (t0, t1) == g_mul_ticks[c]
                if not ACCUM_DMA:
                    op(nc.vector.tensor_tensor(
                        out=locs[:, r0:r1, :, :], in0=locs[:, r0:r1, :, :],
                        in1=tmps[c % 2][:, 0:nr, :, :], op=Alu.add))
                    vwait(state['v'])
                    m_off = ndl[0]['c_M0']
                    assert all(nd['c_M0'] == m_off + 9 * i
                               for i, nd in enumerate(ndl))
                    op(nc.vector.tensor_tensor(
                        out=locs[:, r0:r1, :, :], in0=locs[:, r0:r1, :, :],
                        in1=cbt[:, m_off:m_off + 9 * nr]
                            .rearrange("p (g c) -> p g c", c=9)
                            .unsqueeze(3).broadcast_to([P, nr, 9, Q]),
                        op=Alu.add))
                    vwait(state['v'])
            assert state['v'] == giants_end

            # --- per-node materialize / compose ---
            def r34(t):
                return t[:].rearrange("p (i j) q -> p i j q", j=4)

            def rot_view(t):
                # (P,12,Q) pose tile -> rotation comps (P,3,3,Q)
                return r34(t)[:, :, 0:3, :]

            def pcol(t, k):
                # parent rotation column k broadcast over j: (P,3,3,Q)
                return r34(t)[:, :, k, :].unsqueeze(2).broadcast_to([P, 3, 3, Q])

            def ptrans(t):
                return r34(t)[:, :, 3, :]

            def cbrow(o, n):
                return cbt[:, o:o + n]

            for nd in seq:
                b = nd['buf']
                po = poses[b]
                pu = prior_uses(nd)
                if pu:
                    vector.wait_ge(pd_sems[b], 16 * pu)
                if nd['kind'] == 'mat':
                    if nd.get('trig'):
                        vector.wait_ge(a_sem, 3 * n_groups)
                        t = nd['t']
                        sb_ = sinv[:, t, :].unsqueeze(1).unsqueeze(2) \
                            .broadcast_to([P, 3, 3, Q])
                        cb_ = cosv[:, t, :].unsqueeze(1).unsqueeze(2) \
                            .broadcast_to([P, 3, 3, Q])

                        def c33(o):
                            return cbrow(o, 9) \
                                .rearrange("p (i j) -> p i j", j=3) \
                                .unsqueeze(3).broadcast_to([P, 3, 3, Q])

                        rv = rot_view(po)
                        tA4 = tA[:].rearrange("p (i j) q -> p i j q", j=3)
                        op(nc.vector.tensor_tensor(out=rv, in0=sb_,
                                                   in1=c33(nd['c_B']),
                                                   op=Alu.mult))
                        vwait(state['v'])
                        op(nc.vector.tensor_tensor(out=tA4, in0=cb_,
                                                   in1=c33(nd['c_C']),
                                                   op=Alu.mult))
                        vwait(state['v'])
                        op(nc.vector.tensor_tensor(out=rv, in0=rv, in1=tA4,
                                                   op=Alu.add))
                        vwait(state['v'])
                        op(nc.vector.tensor_tensor(out=rv, in0=rv,
                                                   in1=c33(nd['c_M0']),
                                                   op=Alu.add))
                        vwait(state['v'])
                        # constant translation column
                        op(nc.vector.tensor_copy(
                            out=ptrans(po),
                            in_=cbrow(nd['c_t'], 3).unsqueeze(2)
                                .broadcast_to([P, 3, Q])))
                        vwait(state['v'])
                    else:
                        op(nc.vector.tensor_copy(
                            out=rot_view(po),
                            in_=cbrow(nd['c_R'], 9)
                                .rearrange("p (i j) -> p i j", j=3)
                                .unsqueeze(3).broadcast_to([P, 3, 3, Q])))
                        vwait(state['v'])
                        vector.wait_ge(a_sem, a_u[nd['l']])
                        for k in range(3):
                            op(nc.vector.tensor_copy(
                                out=r34(po)[:, k, 3, :],
                                in_=ut[:, 3 * nd['l'] + k, :]))
                            vwait(state['v'])
                    assert state['v'] == nd['v_done']
                    continue

                # compose: parent pose tile
                pnd = frame_to_node[nd['parent']]
                pp = poses[pnd['buf']]
                if nd['kind'] == 'comp' and nd.get('trig'):
                    if ACCUM_DMA:
                        vector.wait_ge(g_sem, 32 * (nd['chunk'] + 1))
                    g = nd['g']
                    lrow = lambda k, g=g: locs[:, g, 3 * k:3 * k + 3, :] \
                        .unsqueeze(1).broadcast_to([P, 3, 3, Q])
                else:
                    o = nd['c_R']
                    lrow = lambda k, o=o: cbrow(o + 3 * k, 3) \
                        .unsqueeze(1).unsqueeze(3).broadcast_to([P, 3, 3, Q])
                rv = rot_view(po)
                op(nc.vector.tensor_tensor(out=rv, in0=pcol(pp, 0), in1=lrow(0),
                                           op=Alu.mult))
                vwait(state['v'])
                op(nc.vector.tensor_tensor(
                    out=tA[:].rearrange("p (i j) q -> p i j q", j=3),
                    in0=pcol(pp, 1), in1=lrow(1), op=Alu.mult))
                vwait(state['v'])
                op(nc.vector.tensor_tensor(
                    out=tB[:].rearrange("p (i j) q -> p i j q", j=3),
                    in0=pcol(pp, 2), in1=lrow(2), op=Alu.mult))
                vwait(state['v'])
                op(nc.vector.tensor_tensor(
                    out=rv, in0=rv,
                    in1=tA[:].rearrange("p (i j) q -> p i j q", j=3), op=Alu.add))
                vwait(state['v'])
                op(nc.vector.tensor_tensor(
                    out=rv, in0=rv,
                    in1=tB[:].rearrange("p (i j) q -> p i j q", j=3), op=Alu.add))
                vwait(state['v'])
                # translation
                if nd['kind'] == 'comp' and not nd.get('trig', True):
                    # prismatic: u rows are per-batch tensors
                    vector.wait_ge(a_sem, a_u[nd['l']])
                    l3 = nd['l'] * 3
                    urow = lambda k: ut[:, l3 + k, :].unsqueeze(1) \
                        .broadcast_to([P, 3, Q])
                    ppr = r34(pp)
                    op(nc.vector.tensor_tensor(out=ptrans(po),
                       in0=ppr[:, :, 0, :], in1=urow(0), op=Alu.mult))
                    vwait(state['v'])
                    op(nc.vector.tensor_tensor(out=tA[:, 0:3, :],
                       in0=ppr[:, :, 1, :], in1=urow(1), op=Alu.mult))
                    vwait(state['v'])
                    op(nc.vector.tensor_tensor(out=ptrans(po), in0=ptrans(po),
                       in1=tA[:, 0:3, :], op=Alu.add))
                    vwait(state['v'])
                    op(nc.vector.tensor_tensor(out=tA[:, 0:3, :],
                       in0=ppr[:, :, 2, :], in1=urow(2), op=Alu.mult))
                    vwait(state['v'])
                    op(nc.vector.tensor_tensor(out=ptrans(po), in0=ptrans(po),
                       in1=tA[:, 0:3, :], op=Alu.add))
                    vwait(state['v'])
                    op(nc.vector.tensor_tensor(out=ptrans(po), in0=ptrans(po),
                       in1=ptrans(pp), op=Alu.add))
                    vwait(state['v'])
                else:
                    lt = nd['lt']
                    ppr = r34(pp)
                    op(nc.vector.scalar_tensor_tensor(
                        out=ptrans(po), in0=ppr[:, :, 0, :], scalar=lt[0],
                        in1=ptrans(pp), op0=Alu.mult, op1=Alu.add))
                    vwait(state['v'])
                    op(nc.vector.scalar_tensor_tensor(
                        out=ptrans(po), in0=ppr[:, :, 1, :], scalar=lt[1],
                        in1=ptrans(po), op0=Alu.mult, op1=Alu.add))
                    vwait(state['v'])
                    op(nc.vector.scalar_tensor_tensor(
                        out=ptrans(po), in0=ppr[:, :, 2, :], scalar=lt[2],
                        in1=ptrans(po), op0=Alu.mult, op1=Alu.add))
                    vwait(state['v'])
                assert state['v'] == nd['v_done'], (nd['f'], state['v'], nd['v_done'])

    meta = dict(seq_frames=[nd['f'] for nd in seq], const_pose=const_pose,
                NTC=NTC)
    return nc, cb_row, m0img, meta


def _get_program(inputs):
    key_parts = []
    for name in ("all_axes", "all_origins", "mimic_multipliers", "mimic_offsets",
                 "ctrlable_indices", "mimic_src_indices", "mimic_dst_indices",
                 "joint_types"):
        key_parts.append(np.asarray(inputs[name]).tobytes())
    key = hash(tuple(key_parts))
    if key not in _cache:
        nodes, ok = _plan_host(
            inputs["all_axes"], inputs["all_origins"], inputs["mimic_multipliers"],
            inputs["mimic_offsets"], inputs["ctrlable_indices"],
            inputs["mimic_src_indices"], inputs["mimic_dst_indices"],
            inputs["joint_types"])
        if not ok:
            _cache[key] = None
        else:
            nc, cb_row, m0img, meta = _build_program(nodes)
            cb_arr = np.tile(cb_row[None, :], (P, 1))
            m0_arr = np.ascontiguousarray(
                np.tile(m0img.reshape(1, -1), (P, 1)))
            _cache[key] = (nc, cb_arr, m0_arr, meta)
    return _cache[key]


def kernel(**inputs):
    ja = np.ascontiguousarray(np.asarray(inputs["joint_angles"], np.float32))
    assert ja.shape == (B, DOF)
    prog = _get_program(inputs)
    if prog is None:  # non-affine origins: not expected for this problem
        raise RuntimeError("fast path assumptions violated")
    nc, cb_arr, m0_arr, meta = prog
    # host pre-transpose: per core (BC,29) -> (P, 29, Q)
    ja_c = ja.reshape(NCORES, P, Q, DOF).transpose(0, 1, 3, 2)
    in_maps = [{"ja": np.ascontiguousarray(ja_c[c]), "cb": cb_arr,
                "m0img": m0_arr} for c in range(NCORES)]
    res = run_bass_kernel_spmd(nc, in_maps, list(range(NCORES))).results
    full = np.stack([r["out"] for r in res])   # (8, nseq, P, 12*Q)
    nseq = full.shape[1]
    # (8, nseq, P, 12, Q) -> (8, P, Q, nseq, 12) -> (B, nseq, 3, 4)
    full = full.reshape(NCORES, nseq, P, 12, Q).transpose(0, 2, 4, 1, 3)
    rows = np.ascontiguousarray(full).reshape(B, nseq, 3, 4)
    out = np.empty((B, NFRAMES, 4, 4), np.float32)
    out[:, :, 3, :] = np.array([0, 0, 0, 1], np.float32)
    for i, f in enumerate(meta['seq_frames']):
        out[:, f, 0:3, :] = rows[:, i]
    for f, M in meta['const_pose'].items():
        out[:, f] = np.asarray(M, np.float32)[None]
    return out
